# revision 14
# baseline (speedup 1.0000x reference)
"""Trainium2 Bass kernel for nn_DeepRNN: 3-layer LSTM (B=64,T=512,IN=512,H=1024) + FC(4096).

Strategy: tensor-parallel over the 4H gate dimension across 8 cores (each core
owns a 128-wide H-shard and computes the 4 gates for it), with a per-step
all-gather of the hidden state. The 3 layers run as a wavefront (layer l lags
3l ticks) so their per-step work overlaps. Matmuls in bf16 (fp32 PSUM
accumulate), cell state in fp32.

Per-core layouts:
  - gates psum [128(tok = 2 ticks x 64B), 512] = [i|f|o|g] x 128 cols,
    accumulating input-proj (2-tick batch, M=128) + bias + recurrent matmul
    (per tick, M=64 at row offset (t%2)*64).
  - lhsT for all matmuls = transposed activations hT [128(H-chunk), 64(B)]
    obtained from the all-gather via transposing DMA loads into an SBUF ring.
  - weights resident in SBUF as rhs [128(K-chunk), 512] bf16 tiles.
"""
import os
import numpy as np

import concourse.bass as bass
import concourse.bacc as bacc
import concourse.mybir as mybir
from concourse import tile
from concourse.bass_utils import run_bass_kernel_spmd

try:
    from ml_dtypes import bfloat16 as np_bf16
except ImportError:  # pragma: no cover
    import jax.numpy as jnp
    np_bf16 = jnp.bfloat16

N_CORES = 8
B, IN, H, L, OUT = 64, 512, 1024, 3, 4096
HS = H // N_CORES          # 128  per-core H shard
GS = 4 * HS                # 512  per-core gate shard (i|f|o|g)
LAG = 3                    # wavefront lag between layers
RING = 4                   # hT ring depth (+1 replica slot for wraparound pairs)
F32, BF16 = mybir.dt.float32, mybir.dt.bfloat16
AF = mybir.ActivationFunctionType

_LAST_RESULTS = {}


def _build(nc, T):
    SLOT = 3 * GS  # ring slot free-size (3 layers x [128, 512] bf16)

    xT = nc.dram_tensor("xT", [128, 4 * T * B], BF16, kind="ExternalInput")
    wihT = nc.dram_tensor("wihT", [128, 20 * GS], BF16, kind="ExternalInput")
    whhT = nc.dram_tensor("whhT", [128, 24 * GS], BF16, kind="ExternalInput")
    fcwT = nc.dram_tensor("fcwT", [128, 8 * GS], BF16, kind="ExternalInput")
    # biases: [b0|b1|b2|bfc] each GS wide, then 128 ones
    biases = nc.dram_tensor("biases", [1, 4 * GS + 128], BF16, kind="ExternalInput")
    out = nc.dram_tensor("out", [T * B, GS], F32, kind="ExternalOutput")
    debug = os.environ.get("KERNEL_DEBUG", "0") == "1"
    dbg = (nc.dram_tensor("dbg", [512, 3 * HS], F32, kind="ExternalOutput")
           if debug else None)

    xT_v = xT.ap().rearrange("p (k t) -> p k t", k=4)
    xsb_cache = {}

    with tile.TileContext(nc) as tc:
        with (
            tc.tile_pool(name="consts", bufs=1) as cpool,
            tc.tile_pool(name="state", bufs=1) as spool,
            tc.tile_pool(name="xin", bufs=3) as xpool,
            tc.tile_pool(name="tmp", bufs=2) as tpool,
            tc.tile_pool(name="gps", bufs=2, space="PSUM") as gpspool,
            tc.tile_pool(name="fcps", bufs=2, space="PSUM") as fcpool,
            tc.tile_pool(name="outsb", bufs=2) as opool,
            tc.tile_pool(name="dram", bufs=3, space="DRAM") as dpool,
        ):
            # ---- resident weights / constants ----
            wih_sb = cpool.tile([128, 20 * GS], BF16, name="wih_sb")
            whh_sb = cpool.tile([128, 24 * GS], BF16, name="whh_sb")
            fcw_sb = cpool.tile([128, 8 * GS], BF16, name="fcw_sb")
            bias_sb = cpool.tile([1, 4 * GS + 128], BF16, name="bias_sb")
            nc.sync.dma_start(wih_sb[:], wihT.ap())
            nc.sync.dma_start(whh_sb[:], whhT.ap())
            nc.sync.dma_start(fcw_sb[:], fcwT.ap())
            nc.sync.dma_start(bias_sb[:], biases.ap())
            ones_ap = bias_sb[:1, 4 * GS:4 * GS + 128]

            def wih_tile(l, k):  # L0: k=0..3, L1: k=0..7, L2: k=0..7
                base = [0, 4, 12][l] + k
                return wih_sb[:, base * GS:(base + 1) * GS]

            def whh_tile(l, k):
                return whh_sb[:, (8 * l + k) * GS:(8 * l + k + 1) * GS]

            # ---- persistent state ----
            # ringS (slot-major): transpose-DMA dst, feeds single-tick recurrent
            # lhsT reads. col = (l*2 + slot)*512 + k*64 + b.
            # ringK (k-major): scatter dst, feeds 2-tick proj/FC lhsT reads.
            # col = l*LSTRIDE + k*KSTRIDE + slot*64 + b, so (slot, slot+1) for
            # one (l, k) is a contiguous [128, 128] lhsT.
            KSTRIDE = (RING + 1) * 64
            LSTRIDE = 8 * KSTRIDE
            ringS = spool.tile([128, 6 * GS], BF16, name="ringS")
            ring = spool.tile([128, 3 * LSTRIDE], BF16, name="ring")
            c_st = [[spool.tile([64, HS], F32, name=f"c{l}_{p}") for p in range(2)]
                    for l in range(L)]
            h_all = spool.tile([64, 3 * HS], BF16, name="h_all")
            ifo = [spool.tile([64, 3 * HS], F32, name=f"ifo{l}") for l in range(L)]
            g_t = [spool.tile([64, HS], F32, name=f"g{l}") for l in range(L)]
            tc_t = [spool.tile([64, HS], F32, name=f"tc{l}") for l in range(L)]

            def lhs1(s_tick, l, k):  # [128, 64] single tick, from ringS
                off = (l * 2 + s_tick % 2) * GS + k * 64
                return ringS[:, off:off + 64]

            def lhs2(slot, l, k):  # [128, 128] two adjacent ring slots
                off = l * LSTRIDE + k * KSTRIDE + slot * 64
                return ring[:, off:off + 128]

            def ring_dst(slot, l):  # 3D scatter AP [128, k=8, 64] for gather load
                v = ring[:].rearrange("p (l k f) -> p l k f", l=3, k=8)
                return v[:, l, :, slot * 64:(slot + 1) * 64]

            def prefetch_x(t):
                xsb = xpool.tile([128, 512], BF16, name="xsb")
                nc.sync.dma_start(
                    xsb[:].rearrange("p (k t) -> p k t", k=4),
                    xT_v[:, :, t * 64:(t + 2) * 64])
                xsb_cache[t] = xsb

            prefetch_x(0)
            gates_ps = {}

            for s in range(T + 3 * LAG + 2):
                for l in range(L):
                    t = s - LAG * l
                    if not (0 <= t < T):
                        continue
                    grp = (l, t // 2)
                    if t % 2 == 0:
                        # 2-tick psum group: input projection + bias
                        ps = gpspool.tile([128, GS], F32, name=f"ps{l}", tag=f"ps{l}")
                        gates_ps[grp] = ps
                        nk = 4 if l == 0 else 8
                        for k in range(nk):
                            if l == 0:
                                lhs = xsb_cache[t][:, k * 128:(k + 1) * 128]
                            else:
                                lhs = lhs2((t + LAG * (l - 1)) % RING, l - 1, k)
                            nc.tensor.matmul(ps[:], lhs, wih_tile(l, k),
                                             start=(k == 0), stop=False)
                        nc.tensor.matmul(
                            ps[:], ones_ap, bias_sb[:1, l * GS:(l + 1) * GS],
                            start=False, stop=False)
                    ps = gates_ps[grp]
                    r0 = (t % 2) * 64
                    pr = ps[r0:r0 + 64, :]
                    if t > 0:
                        for k in range(8):
                            nc.tensor.matmul(
                                pr, lhs1(s - 1, l, k), whh_tile(l, k),
                                start=False, stop=(k == 7),
                                tile_position=(0, r0) if r0 else None)
                    # ---- gate tail: c = sig(f)*c + sig(i)*tanh(g); h = sig(o)*tanh(c)
                    nc.scalar.activation(ifo[l][:], pr[:, 0:384], AF.Sigmoid)
                    nc.scalar.activation(g_t[l][:], pr[:, 384:512], AF.Tanh)
                    c_new, c_old = c_st[l][t % 2], c_st[l][1 - t % 2]
                    if t > 0:
                        t1 = tpool.tile([64, HS], F32, name=f"t1{l}", tag=f"t1{l}")
                        t2 = tpool.tile([64, HS], F32, name=f"t2{l}", tag=f"t2{l}")
                        nc.vector.tensor_mul(t1[:], ifo[l][:, 128:256], c_old[:])
                        nc.vector.tensor_mul(t2[:], ifo[l][:, 0:128], g_t[l][:])
                        nc.vector.tensor_add(c_new[:], t1[:], t2[:])
                    else:
                        nc.vector.tensor_mul(c_new[:], ifo[l][:, 0:128], g_t[l][:])
                    nc.scalar.activation(tc_t[l][:], c_new[:], AF.Tanh)
                    nc.vector.tensor_mul(h_all[:, l * HS:(l + 1) * HS],
                                         ifo[l][:, 256:384], tc_t[l][:])

                # x prefetch for L0 projection, 2 ticks ahead
                tpre = s + 2
                if tpre % 2 == 0 and tpre < T:
                    prefetch_x(tpre)

                # ---- all-gather of h (batched over layers) ----
                if s <= T - 1 + 2 * LAG:
                    agin = dpool.tile([64, 3 * HS], BF16, name="agin")
                    agout = dpool.tile([64 * N_CORES, 3 * HS], BF16,
                                       name="agout", addr_space="Shared")
                    nc.sync.dma_start(agin[:], h_all[:])
                    nc.gpsimd.collective_compute(
                        "AllGather", mybir.AluOpType.bypass,
                        replica_groups=[list(range(N_CORES))],
                        ins=[agin[:]], outs=[agout[:]])
                    if dbg is not None and s == int(os.environ.get("KERNEL_DEBUG_TICK", "0")):
                        nc.gpsimd.dma_start(dbg.ap(), agout[:])
                    slot = s % RING
                    for l in range(L):
                        stage = ringS[:, (l * 2 + s % 2) * GS:(l * 2 + s % 2 + 1) * GS]
                        nc.sync.dma_start_transpose(
                            stage, agout[:, l * HS:(l + 1) * HS])
                        # off-critical-path scatter into the k-major ring
                        nc.sync.dma_start(ring_dst(slot, l), stage)
                        if slot == 0:  # replica for wraparound slot pairs (3,4)
                            nc.sync.dma_start(ring_dst(RING, l), stage)

                # ---- FC head on h2, 2-tick batches ----
                tf = s - 3 * LAG
                if tf >= 0 and tf % 2 == 0 and tf < T:
                    fps = fcpool.tile([128, GS], F32, name="fps", tag="fps")
                    slot = (tf + 2 * LAG) % RING
                    for k in range(8):
                        nc.tensor.matmul(fps[:], lhs2(slot, 2, k),
                                         fcw_sb[:, k * GS:(k + 1) * GS],
                                         start=(k == 0), stop=False)
                    nc.tensor.matmul(fps[:], ones_ap, bias_sb[:1, 3 * GS:4 * GS],
                                     start=False, stop=True)
                    osb = opool.tile([128, GS], F32, name="osb", tag="osb")
                    nc.scalar.copy(osb[:], fps[:])
                    nc.sync.dma_start(out.ap()[tf * 64:(tf + 2) * 64, :], osb[:])

    return nc


def _prep_core_inputs(inputs, core, T):
    """Host-side shard / gate-reorder / transpose for one core."""
    k = core

    def gate_rows(W):  # rows [i | f | o | g] of this core's H-shard; W [4H, ...]
        return np.concatenate(
            [W[0 * H + k * HS:0 * H + (k + 1) * HS],
             W[1 * H + k * HS:1 * H + (k + 1) * HS],
             W[3 * H + k * HS:3 * H + (k + 1) * HS],
             W[2 * H + k * HS:2 * H + (k + 1) * HS]], axis=0)

    def as_ktiles(WT):  # [K, GS] -> [128, (K/128)*GS], K-chunk-major columns
        K = WT.shape[0]
        return np.ascontiguousarray(
            WT.reshape(K // 128, 128, GS).transpose(1, 0, 2).reshape(128, -1))

    wih_parts, whh_parts, bias_parts = [], [], []
    for l in range(L):
        Wg = gate_rows(np.asarray(inputs[f"Wih{l}"], dtype=np.float32))
        wih_parts.append(as_ktiles(np.ascontiguousarray(Wg.T)))
        Hg = gate_rows(np.asarray(inputs[f"Whh{l}"], dtype=np.float32))
        whh_parts.append(as_ktiles(np.ascontiguousarray(Hg.T)))
        b = (np.asarray(inputs[f"bih{l}"], dtype=np.float32)
             + np.asarray(inputs[f"bhh{l}"], dtype=np.float32))
        bias_parts.append(gate_rows(b[:, None])[:, 0])
    fcW = np.asarray(inputs["fcW"], dtype=np.float32)[k * GS:(k + 1) * GS]
    fcb = np.asarray(inputs["fcb"], dtype=np.float32)[k * GS:(k + 1) * GS]

    x = np.asarray(inputs["x"], dtype=np.float32)[:, :T, :]
    xT = np.ascontiguousarray(
        x.transpose(2, 1, 0).reshape(IN, T * B)      # [IN, t*B + b]
        .reshape(4, 128, T * B).transpose(1, 0, 2).reshape(128, 4 * T * B))

    bias_vec = np.concatenate(bias_parts + [fcb, np.ones(128, np.float32)])
    return {
        "xT": xT.astype(np_bf16),
        "wihT": np.concatenate(wih_parts, axis=1).astype(np_bf16),
        "whhT": np.concatenate(whh_parts, axis=1).astype(np_bf16),
        "fcwT": as_ktiles(np.ascontiguousarray(fcW.T)).astype(np_bf16),
        "biases": bias_vec[None, :].astype(np_bf16),
    }


def kernel(**inputs):
    T = inputs["x"].shape[1]
    nc = bacc.Bacc("TRN2", target_bir_lowering=False, debug=False,
                   num_devices=N_CORES)
    _build(nc, T)
    nc.compile()

    in_maps = [_prep_core_inputs(inputs, c, T) for c in range(N_CORES)]
    trace = os.environ.get("KERNEL_TRACE", "1") == "1"
    res = run_bass_kernel_spmd(nc, in_maps, core_ids=list(range(N_CORES)),
                               trace=trace)
    _LAST_RESULTS["exec_time_ns"] = res.exec_time_ns
    _LAST_RESULTS["res"] = res

    parts = [np.asarray(res.results[c]["out"]) for c in range(N_CORES)]
    full = np.concatenate(parts, axis=1)              # [T*B, 4096], row = t*B+b
    return np.ascontiguousarray(
        full.reshape(T, B, OUT).transpose(1, 0, 2)).astype(np.float32)
